# revision 29
# baseline (speedup 1.0000x reference)
"""Grouped-query attention (GQA) Trainium2 Bass kernel — optimized v2.

Problem: B=2, S=2048, DIM=2048, HQ=32, HKV=8, HEAD_DIM=64, causal mask.
Sharding: 8 cores = 2 (batch) x 4 (kv-head groups). Core c handles batch
c//4 and kv-block c%4 (2 kv heads, 8 q heads). Wq/Wk/Wv sharded
column-wise, Wo row-wise; each core writes a bf16 [S, DIM] partial;
host sums the 4 partials per batch and adds bo.

Key layout decisions (vs v1):
  - Host pre-casts q/k/v to bf16 AND pre-transposes to [DIM, S], so the
    kernel loads [d, s] tiles directly: no on-chip fp32->bf16 casts and
    no PE transposes for q/k/v. Weights are uploaded bf16.
  - Head pairs (p, p+4) are laid out on disjoint 64-partition halves
    (qxT tile p / kxT / attnT), so the two K=64 score matmuls of a pair
    auto-derive tile_position (0,0) / (64,0) and run CONCURRENTLY on
    disjoint PE row-groups.
  - The exp (ACT) is software-pipelined one j-block behind scores, and
    the previous chunk's output projection (GEMM4) is drip-fed into the
    attention instruction stream so PE has fill work during exp waits.
  - Flash-style ones-column in the packed v tile gives the softmax
    denominator for free in the attnV accumulation (row 64).
  - Partial outputs written bf16 (summed fp32 on host with bo).
"""

import numpy as np
import ml_dtypes

import concourse.bass as bass
import concourse.mybir as mybir
from concourse import bacc
from concourse.tile import TileContext
from concourse.bass_utils import run_bass_kernel_spmd

F32 = mybir.dt.float32
BF16 = mybir.dt.bfloat16
AF = mybir.ActivationFunctionType
ALU = mybir.AluOpType

B, S, DIM = 2, 2048, 2048
HQ, HKV, HD = 32, 8, 64
GROUP = HQ // HKV              # 4
NCORES = 8
KVSH = 4                       # kv-blocks (shards) per batch
CQ = (HQ // KVSH) * HD         # 512 q-proj cols per core (8 heads)
CK = (HKV // KVSH) * HD        # 128 kv-proj cols per core (2 heads)
NDC = DIM // 128               # 16 contraction chunks
NSS = S // 512                 # 4 sequence chunks of 512


def _dc_ap(t, ncols, nchunks, w, s0=0):
    """AP viewing DRAM [nrows, ncols] as a [128, nchunks*w] tile:
    (p, dc*w + j) = t[dc*128 + p, s0 + j]."""
    base = t[0:1, 0:1]
    return bass.AP(tensor=base.tensor, offset=s0,
                   ap=[[ncols, 128], [128 * ncols, nchunks], [1, w]])


def _bcast_ap(ap, n):
    """Broadcast a [1, F] AP across n partitions (stride-0 partition)."""
    return bass.AP(tensor=ap.tensor, offset=ap.offset,
                   ap=[[0, n]] + list(ap.ap[1:]))


def build_nc(mode="causal"):
    nc = bacc.Bacc("TRN2", target_bir_lowering=False)

    qt = nc.dram_tensor("qt", [DIM, S], BF16, kind="ExternalInput")
    kt = nc.dram_tensor("kt", [DIM, S], BF16, kind="ExternalInput")
    vt = nc.dram_tensor("vt", [DIM, S], BF16, kind="ExternalInput")
    wq = nc.dram_tensor("wq", [DIM, CQ], BF16, kind="ExternalInput")
    wk = nc.dram_tensor("wk", [DIM, CK], BF16, kind="ExternalInput")
    wv = nc.dram_tensor("wv", [DIM, CK], BF16, kind="ExternalInput")
    wo = nc.dram_tensor("wo", [CQ, DIM], BF16, kind="ExternalInput")
    bq = nc.dram_tensor("bq", [CQ], F32, kind="ExternalInput")
    bk = nc.dram_tensor("bk", [CK], F32, kind="ExternalInput")
    bv = nc.dram_tensor("bv", [CK], F32, kind="ExternalInput")
    tri = nc.dram_tensor("tri", [128, 128], BF16, kind="ExternalInput")
    ident = nc.dram_tensor("ident", [128, 128], BF16, kind="ExternalInput")
    mbias = None
    if mode == "dense":
        mbias = nc.dram_tensor("mbias", [S, S], F32, kind="ExternalInput")
    out = nc.dram_tensor("out", [S, DIM], BF16, kind="ExternalOutput")

    njb_all = S // 128

    from contextlib import ExitStack
    with TileContext(nc) as tc:
        with ExitStack() as _stk:
            def _pool(**kw):
                return _stk.enter_context(tc.tile_pool(**kw))

            consts = _pool(name="consts", bufs=1)
            wpool = _pool(name="w", bufs=1)
            kvp = _pool(name="kv", bufs=6)
            qp = _pool(name="qs", bufs=4)
            acts = _pool(name="acts", bufs=1)
            qxp = _pool(name="qx", bufs=2)
            atnp = _pool(name="atn", bufs=2)
            expp = _pool(name="exp", bufs=6)
            vsbp = _pool(name="vsb", bufs=2)
            atsp = _pool(name="ats", bufs=8)
            nmp = _pool(name="nm", bufs=6)
            rbp = _pool(name="rb", bufs=2)
            mbp = _pool(name="mb", bufs=2)
            obp = _pool(name="ob", bufs=2)
            drp = _pool(name="dr", bufs=4, space="DRAM")
            psc = _pool(name="psc", bufs=2, space="PSUM")
            pat = _pool(name="pat", bufs=2, space="PSUM")
            psg = _pool(name="psg", bufs=2, space="PSUM")
            # ---- constants ----
            tri_t = consts.tile([128, 128], BF16, tag="tri")
            nc.gpsimd.dma_start(out=tri_t[:, :], in_=tri[:, :])
            id_t = consts.tile([128, 128], BF16, tag="id")
            nc.gpsimd.dma_start(out=id_t[:, :], in_=ident[:, :])
            bq_t = consts.tile([128, 4], F32, tag="bq")
            nc.gpsimd.dma_start(
                out=bq_t[:, :],
                in_=bass.AP(tensor=bq[0:1].tensor, offset=0,
                            ap=[[1, 128], [128, 4]]))
            bk_t = consts.tile([128, 1], F32, tag="bk")
            nc.gpsimd.dma_start(
                out=bk_t[:, :],
                in_=bass.AP(tensor=bk[0:1].tensor, offset=0,
                            ap=[[1, 128], [128, 1]]))
            ones1 = consts.tile([1, 128], BF16, tag="ones1")
            nc.vector.memset(ones1[:, :], 1.0)
            bv_rep = consts.tile([128, 128], F32, tag="bv")
            nc.gpsimd.dma_start(
                out=bv_rep[:, :],
                in_=bass.AP(tensor=bv[0:1].tensor, offset=0,
                            ap=[[0, 128], [1, 128]]))

            # ---- weights: bf16 direct loads, ordered by first use ----
            wk_t = wpool.tile([128, NDC * CK], BF16, tag="wk")
            nc.sync.dma_start(out=wk_t[:, :], in_=_dc_ap(wk, CK, NDC, CK))
            # prefetch the first k/v/q token chunks between the weight
            # loads so GEMM1 of chunk 0 starts as early as possible
            kts0a = kvp.tile([128, 8 * 512], BF16, tag="kv")
            nc.sync.dma_start(out=kts0a[:, :], in_=_dc_ap(kt, S, 8, 512, 0))
            kts0b = kvp.tile([128, 8 * 512], BF16, tag="kv")
            nc.sync.dma_start(out=kts0b[:, :],
                              in_=_dc_ap(kt, S, 8, 512, 8 * 128 * S))
            wv_t = wpool.tile([128, NDC * CK], BF16, tag="wv")
            nc.sync.dma_start(out=wv_t[:, :], in_=_dc_ap(wv, CK, NDC, CK))
            vts0a = kvp.tile([128, 8 * 512], BF16, tag="kv")
            nc.sync.dma_start(out=vts0a[:, :], in_=_dc_ap(vt, S, 8, 512, 0))
            vts0b = kvp.tile([128, 8 * 512], BF16, tag="kv")
            nc.sync.dma_start(out=vts0b[:, :],
                              in_=_dc_ap(vt, S, 8, 512, 8 * 128 * S))
            wq_t = wpool.tile([128, NDC * CQ], BF16, tag="wq")
            nc.sync.dma_start(out=wq_t[:, :], in_=_dc_ap(wq, CQ, NDC, CQ))
            qts0a = qp.tile([128, 8 * 512], BF16, tag="qts")
            nc.sync.dma_start(out=qts0a[:, :], in_=_dc_ap(qt, S, 8, 512, 0))
            qts0b = qp.tile([128, 8 * 512], BF16, tag="qts")
            nc.sync.dma_start(out=qts0b[:, :],
                              in_=_dc_ap(qt, S, 8, 512, 8 * 128 * S))
            wo_t = wpool.tile([128, 4 * DIM], BF16, tag="wo")
            nc.sync.dma_start(out=wo_t[:, :], in_=_dc_ap(wo, DIM, 4, DIM))

            # ---- persistent activations ----
            kxT = acts.tile([128, S], BF16, tag="kx", name="kx")
            vx1 = acts.tile([128, 16 * 130], BF16, tag="vx", name="vx")
            # ones columns (flash denominator) in every 130-block at 64/129
            for jb in range(16):
                nc.vector.memset(vx1[:, jb * 130 + 64:jb * 130 + 65], 1.0)
                nc.vector.memset(vx1[:, jb * 130 + 129:jb * 130 + 130], 1.0)

            # GEMM4 of chunk ss-1, drip-fed into chunk ss's attention
            def g4_gen(ss_idx, attnT_s):
                s0p = ss_idx * 512
                for sc in range(4):
                    ob = obp.tile([128, 2048], BF16, tag="ob")
                    for ec in range(4):
                        g4 = psg.tile([128, 512], F32, tag="g")
                        for cc2 in range(4):
                            nc.tensor.matmul(
                                g4[:, :],
                                attnT_s[:, cc2 * 512 + sc * 128:
                                        cc2 * 512 + (sc + 1) * 128],
                                wo_t[:, cc2 * 2048 + ec * 512:
                                     cc2 * 2048 + (ec + 1) * 512],
                                start=(cc2 == 0), stop=(cc2 == 3))
                        nc.vector.tensor_copy(
                            ob[:, ec * 512:(ec + 1) * 512], g4[:, :])
                        yield
                    nc.gpsimd.dma_start(
                        out=out[s0p + sc * 128:s0p + (sc + 1) * 128, :],
                        in_=ob[:, :])

            pending = [None]

            def pump(n):
                if pending[0] is None:
                    return
                for _ in range(n):
                    if next(pending[0], "END") == "END":
                        pending[0] = None
                        break

            for ss in range(NSS):
                s0 = ss * 512

                # ---- k/v chunk (loads were issued an iteration early) ----
                if ss == 0:
                    kh, vh, qh = ([kts0a, kts0b], [vts0a, vts0b],
                                  [qts0a, qts0b])
                else:
                    kh, vh, qh = nxt_tiles
                if ss + 1 < NSS:
                    nxt_tiles = []
                    for src in (kt, vt, qt):
                        pool = qp if src is qt else kvp
                        tg = "qts" if src is qt else "kv"
                        ta = pool.tile([128, 8 * 512], BF16, tag=tg)
                        nc.sync.dma_start(
                            out=ta[:, :], in_=_dc_ap(src, S, 8, 512, s0 + 512))
                        tb = pool.tile([128, 8 * 512], BF16, tag=tg)
                        nc.sync.dma_start(
                            out=tb[:, :],
                            in_=_dc_ap(src, S, 8, 512, 8 * 128 * S + s0 + 512))
                        nxt_tiles.append([ta, tb])
                kts, vts = kh, vh
                ps = psg.tile([128, 512], F32, tag="g")
                for dc in range(NDC):
                    nc.tensor.matmul(ps[:, :], wk_t[:, dc * CK:(dc + 1) * CK],
                                     kts[dc // 8][:, (dc % 8) * 512:
                                                  (dc % 8 + 1) * 512],
                                     start=(dc == 0), stop=(dc == NDC - 1))
                nc.scalar.activation(kxT[:, s0:s0 + 512], ps[:, :],
                                     AF.Identity, bias=bk_t[:, 0:1])
                ps = psg.tile([128, 512], F32, tag="g")
                for dc in range(NDC):
                    nc.tensor.matmul(ps[:, :], wv_t[:, dc * CK:(dc + 1) * CK],
                                     vts[dc // 8][:, (dc % 8) * 512:
                                                  (dc % 8 + 1) * 512],
                                     start=(dc == 0), stop=(dc == NDC - 1))
                vxsb = vsbp.tile([128, 512], BF16, tag="vsb")
                nc.scalar.activation(vxsb[:, :], ps[:, :], AF.Copy)
                vtp = psg.tile([128, 512], BF16, tag="g")
                for sc2 in range(4):
                    nc.tensor.transpose(vtp[:, sc2 * 128:(sc2 + 1) * 128],
                                        vxsb[:, sc2 * 128:(sc2 + 1) * 128],
                                        id_t[:, :])
                for sc2 in range(4):
                    jb = ss * 4 + sc2
                    for h2 in range(2):
                        nc.vector.tensor_tensor(
                            vx1[:, jb * 130 + h2 * 65:jb * 130 + h2 * 65 + 64],
                            vtp[:, sc2 * 128 + h2 * 64:
                                sc2 * 128 + (h2 + 1) * 64],
                            bv_rep[:, h2 * 64:(h2 + 1) * 64], ALU.add)

                # ---- q chunk: GEMM1 (bias on DVE) ----
                qts = qh
                qxT_s = qxp.tile([128, 4 * 512], BF16, tag="qx")
                for cc in range(4):
                    ps = psg.tile([128, 512], F32, tag="g")
                    for dc in range(NDC):
                        nc.tensor.matmul(
                            ps[:, :],
                            wq_t[:, dc * CQ + cc * 128:dc * CQ + (cc + 1) * 128],
                            qts[dc // 8][:, (dc % 8) * 512:
                                         (dc % 8 + 1) * 512],
                            start=(dc == 0), stop=(dc == NDC - 1))
                    nc.vector.tensor_scalar_add(
                        qxT_s[:, cc * 512:(cc + 1) * 512], ps[:, :],
                        bq_t[:, cc:cc + 1])

                # ---- attention: head pairs (p, p+4) on PE row halves ----
                attnT_s = atnp.tile([128, 4 * 512], BF16, tag="at")
                njb = 4 * (ss + 1) if mode == "causal" else njb_all
                for p in range(4):
                    atA = pat.tile([65, 512], F32, tag="pat")
                    atB = pat.tile([65, 512], F32, tag="pat")

                    def emit_at(jb, off, N, ex2):
                        nc.tensor.matmul(
                            atA[:, off:512],
                            vx1[:, jb * 130:jb * 130 + 65], ex2[:, 0:N],
                            start=(jb == 0), stop=(jb == njb - 1))
                        nc.tensor.matmul(
                            atB[:, off:512],
                            vx1[:, jb * 130 + 65:jb * 130 + 130],
                            ex2[:, 512:512 + N],
                            start=(jb == 0), stop=(jb == njb - 1))

                    prev_at = None
                    for jb in range(njb):
                        j0 = jb * 128
                        off = max(0, j0 - s0) if mode == "causal" else 0
                        N = 512 - off
                        # both heads' scores in one 2-bank PSUM tile:
                        # head p at cols [0:N], head p+4 at [512:512+N]
                        sc2 = psc.tile([128, 1024], F32, tag="sc")
                        nc.tensor.matmul(
                            sc2[:, 0:N], kxT[0:64, j0:j0 + 128],
                            qxT_s[0:64, p * 512 + off:(p + 1) * 512],
                            start=True, stop=True)
                        nc.tensor.matmul(
                            sc2[:, 512:512 + N], kxT[64:128, j0:j0 + 128],
                            qxT_s[64:128, p * 512 + off:(p + 1) * 512],
                            start=True, stop=True)
                        if mode == "dense":
                            mb = mbp.tile([128, 512], F32, tag="mb")
                            nc.sync.dma_start(
                                out=mb[:, :N],
                                in_=mbias[j0:j0 + 128, s0 + off:s0 + 512])
                            nc.vector.tensor_tensor(sc2[:, 0:N], sc2[:, 0:N],
                                                    mb[:, :N], ALU.add)
                            nc.vector.tensor_tensor(sc2[:, 512:512 + N],
                                                    sc2[:, 512:512 + N],
                                                    mb[:, :N], ALU.add)
                        # one wide exp covering both heads ([2, N] free AP)
                        ex2 = expp.tile([128, 1024], BF16, tag="ex")
                        sc2f = sc2[:, :]
                        ex2f = ex2[:, :]
                        nc.scalar.activation(
                            bass.AP(tensor=ex2f.tensor, offset=ex2f.offset,
                                    ap=[list(ex2f.ap[0]), [512, 2], [1, N]]),
                            bass.AP(tensor=sc2f.tensor, offset=sc2f.offset,
                                    ap=[list(sc2f.ap[0]), [512, 2], [1, N]]),
                            AF.Exp, scale=0.125)
                        if mode == "causal" and j0 >= s0:
                            nc.vector.tensor_tensor(ex2[:, 0:128],
                                                    ex2[:, 0:128],
                                                    tri_t[:, :], ALU.mult)
                            nc.vector.tensor_tensor(ex2[:, 512:640],
                                                    ex2[:, 512:640],
                                                    tri_t[:, :], ALU.mult)
                        if prev_at is not None:
                            emit_at(*prev_at)
                        prev_at = (jb, off, N, ex2)
                        if p >= 1 and jb % 2 == 1:
                            pump(1)
                    emit_at(*prev_at)

                    # move accumulators off PSUM quickly to free the banks
                    atsA = atsp.tile([65, 512], F32, tag="ats")
                    atsB = atsp.tile([65, 512], F32, tag="ats")
                    nc.vector.tensor_copy(atsA[:, :], atA[:, :])
                    nc.vector.tensor_copy(atsB[:, :], atB[:, :])
                    if p == 3:
                        # final pair gates the last GEMM4: avoid the DMA
                        # broadcast chain entirely -- reciprocal as
                        # exp(-ln d) on the (idle) ACT engine, broadcast
                        # across partitions with a K=1 ones matmul on PE
                        rln = rbp.tile([1, 1024], F32, tag="rln")
                        nc.scalar.activation(rln[0:1, 0:512],
                                             atsA[64:65, :], AF.Ln)
                        nc.scalar.activation(rln[0:1, 512:1024],
                                             atsB[64:65, :], AF.Ln)
                        rrc = rbp.tile([1, 1024], BF16, tag="rrc")
                        nc.scalar.activation(rrc[0:1, :], rln[0:1, :],
                                             AF.Exp, scale=-1.0)
                        nmA_ps = psg.tile([128, 512], F32, tag="g")
                        nc.tensor.matmul(nmA_ps[0:64, :],
                                         ones1[0:1, 0:64],
                                         rrc[0:1, 0:512],
                                         start=True, stop=True)
                        nmB_ps = psg.tile([128, 512], F32, tag="g")
                        nc.tensor.matmul(nmB_ps[0:64, :],
                                         ones1[0:1, 0:64],
                                         rrc[0:1, 512:1024],
                                         start=True, stop=True)
                        nc.vector.tensor_tensor(
                            attnT_s[0:64, p * 512:(p + 1) * 512],
                            atsA[0:64, :], nmA_ps[0:64, :], ALU.mult)
                        nc.vector.tensor_tensor(
                            attnT_s[64:128, p * 512:(p + 1) * 512],
                            atsB[0:64, :], nmB_ps[0:64, :], ALU.mult)
                    else:
                        # normalize: the [1,512] denominator rows would use
                        # a single DVE lane (8 cyc/elem reciprocal =
                        # 3.3us!), so bounce through DRAM reshaped to
                        # [128,8] to use all lanes, then broadcast back.
                        dma = nc.gpsimd.dma_start
                        dr = drp.tile([2, 512], F32, tag="dn")
                        dma(out=dr[0:1, :], in_=atsA[64:65, :])
                        dma(out=dr[1:2, :], in_=atsB[64:65, :])
                        drf = dr[0:1, 0:1]
                        sq = rbp.tile([128, 8], F32, tag="rba")
                        dma(out=sq[:, :],
                            in_=bass.AP(tensor=drf.tensor, offset=drf.offset,
                                        ap=[[8, 128], [1, 8]]))
                        sqr = rbp.tile([128, 8], F32, tag="rbb")
                        nc.vector.reciprocal(sqr[:, :], sq[:, :])
                        dr2 = drp.tile([2, 512], F32, tag="dn2")
                        d2f = dr2[0:1, 0:1]
                        dma(out=bass.AP(tensor=d2f.tensor, offset=d2f.offset,
                                        ap=[[8, 128], [1, 8]]),
                            in_=sqr[:, :])
                        nmA = nmp.tile([64, 512], F32, tag="nm")
                        dma(out=nmA[:, :], in_=_bcast_ap(dr2[0:1, :], 64))
                        nmB = nmp.tile([64, 512], F32, tag="nm")
                        dma(out=nmB[:, :], in_=_bcast_ap(dr2[1:2, :], 64))
                        nc.vector.tensor_tensor(
                            attnT_s[0:64, p * 512:(p + 1) * 512],
                            atsA[0:64, :], nmA[:, :], ALU.mult)
                        nc.vector.tensor_tensor(
                            attnT_s[64:128, p * 512:(p + 1) * 512],
                            atsB[0:64, :], nmB[:, :], ALU.mult)

                pump(24)  # finish any GEMM4 leftovers of chunk ss-1 (incl. tail DMAs)
                pending[0] = g4_gen(ss, attnT_s)

            pump(24)  # GEMM4 of the last chunk
    nc.finalize()
    return nc


_CACHE = {}


def _get_nc(mode):
    if mode not in _CACHE:
        _CACHE[mode] = build_nc(mode)
    return _CACHE[mode]


def kernel(q, k, v, mask, Wq, bq, Wk, bk, Wv, bv, Wo, bo):
    bf = ml_dtypes.bfloat16
    q = np.asarray(q, np.float32)
    k = np.asarray(k, np.float32)
    v = np.asarray(v, np.float32)
    mask = np.asarray(mask)
    Wq = np.asarray(Wq, np.float32)
    Wk = np.asarray(Wk, np.float32)
    Wv = np.asarray(Wv, np.float32)
    Wo = np.asarray(Wo, np.float32)
    bq = np.asarray(bq, np.float32)
    bk = np.asarray(bk, np.float32)
    bv = np.asarray(bv, np.float32)
    bo = np.asarray(bo, np.float32)

    m = mask.astype(np.float64)
    if np.array_equal(m, np.tril(np.ones((S, S)))):
        mode = "causal"
    elif np.all(m == 1):
        mode = "none"
    else:
        mode = "dense"

    nc = _get_nc(mode)
    tri_np = np.ascontiguousarray(
        np.triu(np.ones((128, 128))).astype(bf))
    id_np = np.ascontiguousarray(np.eye(128).astype(bf))

    # On-chip layout places local q head h in tile h%4 at partition
    # (h//4)*64 so q/k partition bases match in the scores matmul and the
    # pair (h, h+4) lands on disjoint PE row groups. Permute Wq columns /
    # Wo rows / bq accordingly: tile cc holds heads (cc, cc+4).
    head_perm = [h for cc in range(4) for h in (cc, cc + 4)]
    col_perm = np.concatenate(
        [np.arange(h * HD, (h + 1) * HD) for h in head_perm])

    qT = [np.ascontiguousarray(q[b].astype(bf).T) for b in range(B)]
    kT = [np.ascontiguousarray(k[b].astype(bf).T) for b in range(B)]
    vT = [np.ascontiguousarray(v[b].astype(bf).T) for b in range(B)]

    in_maps = []
    for core in range(NCORES):
        b, kb = core // KVSH, core % KVSH
        wq_sh = Wq[:, kb * CQ:(kb + 1) * CQ][:, col_perm].astype(bf)
        wo_sh = Wo[kb * CQ:(kb + 1) * CQ, :][col_perm, :].astype(bf)
        bq_sh = bq[kb * CQ:(kb + 1) * CQ][col_perm]
        im = {
            "qt": qT[b],
            "kt": kT[b],
            "vt": vT[b],
            "wq": np.ascontiguousarray(wq_sh),
            "wk": np.ascontiguousarray(
                Wk[:, kb * CK:(kb + 1) * CK].astype(bf)),
            "wv": np.ascontiguousarray(
                Wv[:, kb * CK:(kb + 1) * CK].astype(bf)),
            "wo": np.ascontiguousarray(wo_sh),
            "bq": np.ascontiguousarray(bq_sh),
            "bk": np.ascontiguousarray(bk[kb * CK:(kb + 1) * CK]),
            "bv": np.ascontiguousarray(bv[kb * CK:(kb + 1) * CK]),
            "tri": tri_np,
            "ident": id_np,
        }
        if mode == "dense":
            with np.errstate(divide="ignore"):
                bias = -(1.0 / mask.astype(np.float32) + 1.0)
            im["mbias"] = np.ascontiguousarray(bias.T * 8.0)
        in_maps.append(im)

    res = run_bass_kernel_spmd(nc, in_maps, core_ids=list(range(NCORES)))
    outs = [r["out"] for r in res.results]
    full = np.empty((B, S, DIM), np.float32)
    for b in range(B):
        acc = outs[b * KVSH].astype(np.float32)
        for kb in range(1, KVSH):
            acc = acc + outs[b * KVSH + kb].astype(np.float32)
        full[b] = acc + bo[None, :]
    return full
